# revision 1
# baseline (speedup 1.0000x reference)
"""DOFENTransformer Trainium2 kernel.

Data-parallel over batch: 16 batches / 8 cores = 2 per core. Host does only
x-independent weight folding + index gathers (Gram tables, permuted row
matrices, forest masks) and pure re-layout of x; every x-dependent FLOP runs
on device.

Device algorithm (per core, per batch b):
  logits from gathered Gram values  l_t = s*(x0*xt*Gaa + x0*Gac + xt*Gca + Gcc)
  softmax over the 4 rODT tokens -> a; m = a*x_t
  E_in/w_in[seq,:] = x0*RW0 + RB0 + sum_t (m_t*AvR_t + a_t*CvR_t)   (attention
    output + residual, with Wqkv-v folded into gathered row matrices)
  LayerNorm folded into projections: w_out -> expw; E branch folded through
    the forest contraction F^T = WoEg^T@(E_in^T@SMT) + rank-1 corrections
  bagging MLP in [hidden-on-partition, forest-on-free] layout.
"""
import sys

for p in ('/opt/trn_rl_repo', '/root/.axon_site/_ro/trn_rl_repo'):
    if p not in sys.path:
        sys.path.insert(0, p)

import numpy as np
import concourse.bass as bass
import concourse.bacc as bacc_mod
from concourse import mybir
from concourse.tile import TileContext
from concourse.bass_utils import run_bass_kernel_spmd

B, N_COL, N_COND, D, H = 16, 100, 64, 4, 128
N_FOREST, N_CLASS = 100, 10
NSEQ = 1600
NBLK = 13
PAD_SEQ = NBLK * 128
EPS = 1e-5
S128 = float(np.sqrt(128.0))
F32 = mybir.dt.float32
AF = mybir.ActivationFunctionType
OP = mybir.AluOpType
NCORES = 8

BLKW = 2404
TAILW = 662


def _host_precompute(inp):
    sl = lambda i: slice(i * H, (i + 1) * H)
    Wn = inp['W_num'].reshape(N_COND, H).astype(np.float32)
    Bn = inp['b_num'].reshape(N_COND, H).astype(np.float32)
    Wqkv, bqkv = inp['Wqkv'].astype(np.float32), inp['bqkv'].astype(np.float32)
    perm = inp['perm'].astype(np.int64)
    A = Wn @ Wqkv
    C = Bn @ Wqkv + bqkv
    out = {}
    g_of = np.arange(NSEQ) // 64
    j_of = np.arange(NSEQ) % 64
    p_t = np.zeros((PAD_SEQ, D), np.int64)
    for t in range(D):
        p_t[:NSEQ, t] = perm[4 * g_of + t, j_of]
    valid = np.zeros(PAD_SEQ, bool)
    valid[:NSEQ] = True

    gv = np.zeros((PAD_SEQ, 2, 4, D), np.float32)
    for br in range(2):
        Aq, Ak = A[:, sl(3 * br)], A[:, sl(3 * br + 1)]
        Cq, Ck = C[:, sl(3 * br)], C[:, sl(3 * br + 1)]
        G = (Aq @ Ak.T, Aq @ Ck.T, Cq @ Ak.T, Cq @ Ck.T)
        p0 = p_t[:, 0]
        for kind in range(4):
            for t in range(D):
                pt = p_t[:, t]
                gv[valid, br, kind, t] = G[kind][p0[valid], pt[valid]]
    out['gv_rows'] = gv.reshape(NBLK, 128, 2, 4, D).transpose(1, 2, 3, 0, 4).reshape(128, 416).copy()

    for br, name in ((0, 'w'), (1, 'E')):
        WV, bV = Wqkv[:, sl(3 * br + 2)], bqkv[sl(3 * br + 2)]
        AvR = (Wn @ WV)[p_t]
        CvR = ((Bn @ WV) + bV)[p_t]
        AvR[~valid] = 0
        CvR[~valid] = 0
        out[f'AvR_{name}'] = AvR.reshape(NBLK, 128, D, H).transpose(1, 0, 2, 3).reshape(128, NBLK * D * H).copy()
        out[f'CvR_{name}'] = CvR.reshape(NBLK, 128, D, H).transpose(1, 0, 2, 3).reshape(128, NBLK * D * H).copy()
    RW0 = Wn[p_t[:, 0]]
    RB0 = Bn[p_t[:, 0]]
    RW0[~valid] = 0
    RB0[~valid] = 0
    out['RW0'] = RW0.reshape(NBLK, 128, H).transpose(1, 0, 2).reshape(128, NBLK * H).copy()
    out['RB0'] = RB0.reshape(NBLK, 128, H).transpose(1, 0, 2).reshape(128, NBLK * H).copy()

    Wowg = inp['gamma_w'].astype(np.float32) * inp['Wow'][:, 0].astype(np.float32)
    out['Wowg_bc'] = np.tile(Wowg[None, :], (128, 1)).astype(np.float32)
    out['csumw_neg'] = np.full((128, 1), -Wowg.sum(), np.float32)
    out['bow2'] = np.full((128, 1), float(inp['beta_w'] @ inp['Wow'][:, 0] + inp['bow'][0]), np.float32)

    WoEg = inp['gamma_E'].astype(np.float32)[:, None] * inp['WoE'].astype(np.float32)
    out['WoEg'] = WoEg.copy()
    out['csumE_neg'] = -WoEg.sum(0, keepdims=True).astype(np.float32)
    out['boE2'] = (inp['beta_E'] @ inp['WoE'] + inp['boE'])[None, :].astype(np.float32)

    swr = inp['swr'].astype(np.int64)
    M01 = np.zeros((PAD_SEQ, N_FOREST), np.float32)
    for f in range(N_FOREST):
        r = swr[f]
        seq = (r % 25) * 64 + (r // 25)
        M01[seq, f] = 1.0
    out['M01T'] = M01.reshape(NBLK, 128, N_FOREST).transpose(1, 0, 2).reshape(128, NBLK * N_FOREST).copy()

    out['W1p'] = (inp['g1'].astype(np.float32)[:, None] * inp['W1'].astype(np.float32)).copy()
    out['b1p'] = (inp['be1'] @ inp['W1'] + inp['b1'])[:, None].astype(np.float32)
    W2p = inp['g2'].astype(np.float32)[:, None] * inp['W2'].astype(np.float32)
    out['W2p'] = np.concatenate([W2p, np.zeros((H, 6), np.float32)], 1)  # pad 10->16
    b2p = (inp['be2'] @ inp['W2'] + inp['b2']).astype(np.float32)
    out['b2p'] = np.concatenate([b2p, np.zeros(6, np.float32)])[:, None].copy()
    out['ones'] = np.ones((128, 1), np.float32)
    out['eps'] = np.full((128, 1), EPS, np.float32)
    res = {}
    # per-block tiles: AvR_w(512) CvR_w(512) AvR_E(512) CvR_E(512) RW0(128) RB0(128) M01T(100)
    for k in range(NBLK):
        t = np.zeros((128, BLKW), np.float32)
        t[:, 0:512] = out['AvR_w'][:, k * 512:(k + 1) * 512]
        t[:, 512:1024] = out['CvR_w'][:, k * 512:(k + 1) * 512]
        t[:, 1024:1536] = out['AvR_E'][:, k * 512:(k + 1) * 512]
        t[:, 1536:2048] = out['CvR_E'][:, k * 512:(k + 1) * 512]
        t[:, 2048:2176] = out['RW0'][:, k * H:(k + 1) * H]
        t[:, 2176:2304] = out['RB0'][:, k * H:(k + 1) * H]
        t[:, 2304:2404] = out['M01T'][:, k * N_FOREST:(k + 1) * N_FOREST]
        res[f'blk{k}'] = t
    tail = np.zeros((128, TAILW), np.float32)
    tail[:, 0:128] = out['Wowg_bc']
    tail[:, 128:256] = out['WoEg']
    tail[:, 256:384] = out['W1p']
    tail[:, 384:400] = out['W2p']
    tail[:, 400:401] = out['csumw_neg']
    tail[:, 401:402] = out['bow2']
    tail[:, 402:403] = out['b1p']
    tail[:, 403:404] = out['ones']
    tail[:, 404:405] = out['eps']
    tail[0:1, 405:533] = out['csumE_neg']
    tail[0:1, 533:661] = out['boE2']
    tail[0:16, 661:662] = out['b2p']
    res['tail'] = tail
    res['gvh'] = out['gv_rows']
    return res


def _host_x(inp, bs):
    x = inp['x'].astype(np.float32)
    xt = np.zeros((NBLK, 128, 2, D), np.float32)
    x0 = np.zeros((NBLK, 128, 2), np.float32)
    seq = np.arange(NSEQ)
    g_of = seq // 64
    blk, p = seq // 128, seq % 128
    for bi, b in enumerate(bs):
        for t in range(D):
            xt[blk, p, bi, t] = x[b, 4 * g_of + t]
        x0[blk, p, bi] = x[b, 4 * g_of]
    xd = np.zeros((128, 130), np.float32)
    xd[:, 0:104] = xt.transpose(1, 2, 0, 3).reshape(128, 2 * NBLK * D)
    xd[:, 104:130] = x0.transpose(1, 2, 0).reshape(128, 2 * NBLK)
    return {'xd': xd}


# ---------------------------------------------------------------------------

_H_SHAPES = {'gvh': (128, 416), 'tail': (128, TAILW), 'xd': (128, 130),
             **{f'blk{k}': (128, BLKW) for k in range(NBLK)}}


def _pbcast(nc, dst, src_row, dslice):
    """Broadcast a [1, N] SBUF row to [128, N] via a DRAM bounce."""
    nc.gpsimd.dma_start(out=dslice, in_=src_row)
    src = bass.AP(tensor=dslice.tensor, offset=dslice.offset,
                  ap=[[0, dst.shape[0]]] + [list(d) for d in dslice.ap])
    nc.gpsimd.dma_start(out=dst, in_=src)


def _vw1(ap):
    """First-partition [1,1] view of a [P,1] AP."""
    return bass.AP(tensor=ap.tensor, offset=ap.offset, ap=[[ap.ap[0][0], 1]] + [list(d) for d in ap.ap[1:]])


def _vw(ap, off, dims):
    """Strided free-dim view of a tile AP: dims = [(step, count), ...]."""
    return bass.AP(tensor=ap.tensor, offset=ap.offset + off,
                   ap=[list(ap.ap[0])] + [[s, c] for (s, c) in dims])


def _build_nc():
    nc = bacc_mod.Bacc()
    dram = {k: nc.declare_dram_parameter(k, list(sh), F32, isOutput=False)
            for k, sh in _H_SHAPES.items()}
    out_d = nc.declare_dram_parameter('out', [2, 16], F32, isOutput=True)
    bounce = nc.dram_tensor('bounce', [2, 600], F32)

    with TileContext(nc) as tc:
        with (
            tc.tile_pool(name='const', bufs=1) as cp,
            tc.tile_pool(name='stage', bufs=1) as sp,
            tc.tile_pool(name='scratch', bufs=2) as scr,
            tc.tile_pool(name='small', bufs=2) as sm,
            tc.tile_pool(name='psum', bufs=1, space='PSUM') as pp,
            tc.tile_pool(name='psum1', bufs=1, space='PSUM') as pp1,
        ):
            tiles = {}
            for k in _H_SHAPES:
                t = cp.tile(list(_H_SHAPES[k]), F32, tag=k)
                nc.sync.dma_start(out=t[:, :], in_=dram[k][:, :])
                tiles[k] = t
            blkt = [tiles[f'blk{k}'] for k in range(NBLK)]
            tail = tiles['tail']
            sb = {
                'gv_rows': tiles['gvh'][:, :],
                'xt_rows': tiles['xd'][:, 0:104],
                'x0_rows': tiles['xd'][:, 104:130],
                'Wowg_bc': tail[:, 0:128],
                'WoEg': tail[:, 128:256],
                'W1p': tail[:, 256:384],
                'W2p': tail[:, 384:400],
                'csumw_neg': tail[:, 400:401],
                'bow2': tail[:, 401:402],
                'b1p': tail[:, 402:403],
                'ones': tail[:, 403:404],
                'csumE_neg': tail[0:1, 405:533],
                'boE2': tail[0:1, 533:661],
                'b2p': tail[0:16, 661:662],
            }
            eps_sb = tail[:, 404:405]
            touch = cp.tile([128, 1], F32, tag='touch')
            touch_a = cp.tile([128, 1], F32, tag='touch_a')

            def _touch(src):
                nc.vector.tensor_copy(touch[:, 0:1], src)

            nc.scalar.copy(touch_a[:, 0:1], tail[:, 0:1])
            nc.scalar.copy(touch_a[:, 0:1], tiles['xd'][:, 0:1])

            for b in range(2):
                # ---- logits + softmax: [128, (br, blk, t)] = [128, 104]
                gvv = lambda kind: _vw(sb['gv_rows'], kind * 52, [(208, 2), (4, NBLK), (1, 4)])
                xtv = _vw(sb['xt_rows'], b * 52, [(0, 2), (4, NBLK), (1, 4)])
                x0v = _vw(sb['x0_rows'], b * NBLK, [(0, 2), (1, NBLK), (0, 4)])
                t1 = scr.tile([128, 104], F32, tag='t1')
                t2 = scr.tile([128, 104], F32, tag='t2')
                _touch(tiles['xd'][:, 0:1])
                nc.vector.tensor_mul(t1[:, :], gvv(0), xtv)
                nc.vector.tensor_add(t1[:, :], t1[:, :], gvv(1))
                nc.vector.tensor_mul(t1[:, :], t1[:, :], x0v)
                nc.vector.tensor_mul(t2[:, :], gvv(2), xtv)
                nc.vector.tensor_add(t2[:, :], t2[:, :], gvv(3))
                nc.vector.tensor_add(t1[:, :], t1[:, :], t2[:, :])
                e = scr.tile([128, 104], F32, tag='e')
                nc.scalar.activation(e[:, :], t1[:, :], AF.Exp, bias=0.0, scale=S128)
                esum = sm.tile([128, 26], F32, tag='esum')
                nc.vector.tensor_reduce(esum[:, :], _vw(e[:, :], 0, [(4, 26), (1, 4)]),
                                        mybir.AxisListType.X, OP.add)
                nc.vector.reciprocal(esum[:, :], esum[:, :])
                a_t = scr.tile([128, 104], F32, tag='a')
                m_t = scr.tile([128, 104], F32, tag='m')
                nc.vector.tensor_mul(a_t[:, :], e[:, :], _vw(esum[:, :], 0, [(1, 26), (0, 4)]))
                nc.vector.tensor_mul(m_t[:, :], a_t[:, :], xtv)

                # ---- accumulation chains -> E_in / w_in staging [128, 1664]
                stg = {}
                for br, name in ((0, 'w'), (1, 'E')):
                    acc = sp.tile([128, NBLK * H], F32, tag=f'in_{name}')
                    stg[name] = acc
                    for k in range(NBLK):
                        asl = acc[:, k * H:(k + 1) * H]
                        x0c = sb['x0_rows'][:, b * NBLK + k: b * NBLK + k + 1]
                        nc.vector.scalar_tensor_tensor(
                            asl, blkt[k][:, 2048:2176], x0c,
                            blkt[k][:, 2176:2304], OP.mult, OP.add)
                        for t in range(D):
                            mc = m_t[:, br * 52 + k * 4 + t: br * 52 + k * 4 + t + 1]
                            ac = a_t[:, br * 52 + k * 4 + t: br * 52 + k * 4 + t + 1]
                            av = blkt[k][:, br * 1024 + t * H: br * 1024 + (t + 1) * H]
                            cv = blkt[k][:, br * 1024 + 512 + t * H: br * 1024 + 512 + (t + 1) * H]
                            nc.vector.scalar_tensor_tensor(asl, av, mc, asl, OP.mult, OP.add)
                            nc.vector.scalar_tensor_tensor(asl, cv, ac, asl, OP.mult, OP.add)

                # ---- stats (mu, rstd) per branch [128, 13]
                stats = {}
                for name in ('w', 'E'):
                    acc = stg[name]
                    ssum = sm.tile([128, NBLK], F32, tag=f'sum_{name}')
                    nc.vector.tensor_reduce(ssum[:, :], _vw(acc[:, :], 0, [(H, NBLK), (1, H)]),
                                            mybir.AxisListType.X, OP.add)
                    sq = sp.tile([128, NBLK * H], F32, tag='sq')
                    nc.scalar.square(sq[:, :], acc[:, :])
                    ssq = sm.tile([128, NBLK], F32, tag=f'ssq_{name}')
                    nc.vector.tensor_reduce(ssq[:, :], _vw(sq[:, :], 0, [(H, NBLK), (1, H)]),
                                            mybir.AxisListType.X, OP.add)
                    mu = sm.tile([128, NBLK], F32, tag=f'mu_{name}')
                    nc.vector.tensor_scalar_mul(mu[:, :], ssum[:, :], 1.0 / H)
                    var = sm.tile([128, NBLK], F32, tag=f'var_{name}')
                    nc.vector.tensor_scalar_mul(var[:, :], ssq[:, :], 1.0 / H)
                    mu2 = sm.tile([128, NBLK], F32, tag=f'mu2_{name}')
                    nc.vector.tensor_mul(mu2[:, :], mu[:, :], mu[:, :])
                    nc.vector.tensor_sub(var[:, :], var[:, :], mu2[:, :])
                    nc.scalar.activation(var[:, :], var[:, :], AF.Sqrt, bias=eps_sb, scale=1.0)
                    rstd = sm.tile([128, NBLK], F32, tag=f'rstd_{name}')
                    nc.vector.reciprocal(rstd[:, :], var[:, :])
                    stats[name] = (mu, rstd)

                # ---- w branch -> expw [128, 13]
                wraw = sm.tile([128, NBLK], F32, tag='wraw')
                ttr_scr = scr.tile([128, H], F32, tag='ttr')
                _touch(tail[:, 0:1])
                for k in range(NBLK):
                    nc.vector.tensor_mul(ttr_scr[:, :], stg['w'][:, k * H:(k + 1) * H], sb['Wowg_bc'])
                    nc.vector.tensor_reduce(wraw[:, k:k + 1], ttr_scr[:, :],
                                            mybir.AxisListType.X, OP.add)
                mu_w, rstd_w = stats['w']
                o1 = sm.tile([128, NBLK], F32, tag='o1')
                o2 = sm.tile([128, NBLK], F32, tag='o2')
                nc.vector.tensor_mul(o1[:, :], wraw[:, :], rstd_w[:, :])
                nc.vector.tensor_mul(o2[:, :], mu_w[:, :], rstd_w[:, :])
                nc.vector.scalar_tensor_tensor(o2[:, :], o2[:, :], sb['csumw_neg'],
                                               o1[:, :], OP.mult, OP.add)
                expw = sm.tile([128, NBLK], F32, tag='expw')
                nc.scalar.activation(expw[:, :], o2[:, :], AF.Exp,
                                     bias=sb['bow2'], scale=1.0)

                # ---- E forest contraction
                mu_E, rstd_E = stats['E']
                er = sm.tile([128, NBLK], F32, tag='er')
                emr = sm.tile([128, NBLK], F32, tag='emr')
                nc.vector.tensor_mul(er[:, :], expw[:, :], rstd_E[:, :])
                nc.vector.tensor_mul(emr[:, :], er[:, :], mu_E[:, :])
                main_ps = pp.tile([128, N_FOREST], F32, tag='main')
                vz1_ps = pp1.tile([1, N_FOREST], F32, tag='vz1')
                vz2_ps = pp1.tile([1, N_FOREST], F32, tag='vz2')
                for k in range(NBLK):
                    smt = scr.tile([128, N_FOREST], F32, tag='smt')
                    nc.vector.tensor_scalar_mul(smt[:, :], blkt[k][:, 2304:2404],
                                                er[:, k:k + 1])
                    nc.tensor.matmul(main_ps[:, :], stg['E'][:, k * H:(k + 1) * H], smt[:, :],
                                     start=(k == 0), stop=(k == NBLK - 1))
                    m01k = blkt[k][:, 2304:2404]
                    nc.tensor.matmul(vz1_ps[:, :], emr[:, k:k + 1], m01k,
                                     start=(k == 0), stop=(k == NBLK - 1))
                    nc.tensor.matmul(vz2_ps[:, :], expw[:, k:k + 1], m01k,
                                     start=(k == 0), stop=(k == NBLK - 1))
                main_s = scr.tile([128, N_FOREST], F32, tag='main_s')
                nc.scalar.copy(main_s[:, :], main_ps[:, :])
                v2_s = scr.tile([1, N_FOREST], F32, tag='v2_s')
                z_s = scr.tile([1, N_FOREST], F32, tag='z_s')
                nc.scalar.copy(v2_s[:, :], vz1_ps[:, :])
                nc.scalar.copy(z_s[:, :], vz2_ps[:, :])
                ft_ps = pp.tile([128, N_FOREST], F32, tag='ft')
                nc.tensor.matmul(ft_ps[:, :], sb['WoEg'], main_s[:, :], start=True, stop=False)
                nc.tensor.matmul(ft_ps[:, :], sb['csumE_neg'], v2_s[:, :], start=False, stop=False, skip_group_check=True)
                nc.tensor.matmul(ft_ps[:, :], sb['boE2'], z_s[:, :], start=False, stop=True, skip_group_check=True)
                rz = sm.tile([1, N_FOREST], F32, tag='rz')
                nc.vector.reciprocal(rz[:, :], z_s[:, :])
                rz_bc = scr.tile([128, N_FOREST], F32, tag='rz_bc')
                _pbcast(nc, rz_bc[:, :], rz[0:1, :], bounce[b, 0:100])
                F_s = scr.tile([128, N_FOREST], F32, tag='F_s')
                _touch(rz_bc[:, 0:1])
                nc.vector.tensor_mul(F_s[:, :], ft_ps[:, :], rz_bc[:, :])

                # ---- bagging (hidden on partitions, forests on free)
                LNOFF = {'l1': 0, 'l2': 200}

                def ln_cols(V, nm):
                    cs1 = pp1.tile([1, N_FOREST], F32, tag='cs1')
                    cs2 = pp1.tile([1, N_FOREST], F32, tag='cs2')
                    sqv = scr.tile([128, N_FOREST], F32, tag=f'sqv_{nm}')
                    nc.vector.tensor_mul(sqv[:, :], V, V)
                    nc.tensor.matmul(cs1[:, :], sb['ones'], V, start=True, stop=True)
                    nc.tensor.matmul(cs2[:, :], sb['ones'], sqv[:, :], start=True, stop=True)
                    strow = sm.tile([1, 2 * N_FOREST], F32, tag=f'st_{nm}')
                    tmp = sm.tile([1, N_FOREST], F32, tag=f'tmp_{nm}')
                    nc.vector.tensor_scalar_mul(strow[0:1, 0:N_FOREST], cs1[:, :], 1.0 / H)
                    nc.vector.tensor_scalar_mul(strow[0:1, N_FOREST:], cs2[:, :], 1.0 / H)
                    nc.vector.tensor_mul(tmp[:, :], strow[0:1, 0:N_FOREST], strow[0:1, 0:N_FOREST])
                    nc.vector.tensor_sub(strow[0:1, N_FOREST:], strow[0:1, N_FOREST:], tmp[:, :])
                    nc.scalar.activation(strow[0:1, N_FOREST:], strow[0:1, N_FOREST:],
                                         AF.Sqrt, bias=_vw1(eps_sb), scale=1.0)
                    nc.vector.reciprocal(strow[0:1, N_FOREST:], strow[0:1, N_FOREST:])
                    mb = scr.tile([128, 2 * N_FOREST], F32, tag=f'mb_{nm}')
                    _pbcast(nc, mb[:, :], strow[0:1, :], bounce[b, 100 + LNOFF[nm]:300 + LNOFF[nm]])
                    LN = scr.tile([128, N_FOREST], F32, tag=f'ln_{nm}')
                    _touch(mb[:, 0:1])
                    nc.vector.tensor_sub(LN[:, :], V, mb[:, 0:N_FOREST])
                    nc.vector.tensor_mul(LN[:, :], LN[:, :], mb[:, N_FOREST:2 * N_FOREST])
                    return LN

                LN1 = ln_cols(F_s[:, :], 'l1')
                h1_ps = pp.tile([128, N_FOREST], F32, tag='h1')
                nc.tensor.matmul(h1_ps[:, :], sb['W1p'], LN1[:, :], start=True, stop=True)
                h1 = scr.tile([128, N_FOREST], F32, tag='h1s')
                nc.scalar.activation(h1[:, :], h1_ps[:, :], AF.Relu,
                                     bias=sb['b1p'], scale=1.0)
                LN2 = ln_cols(h1[:, :], 'l2')
                o_ps = pp.tile([16, N_FOREST], F32, tag='ops')
                nc.tensor.matmul(o_ps[:, :], sb['W2p'], LN2[:, :], start=True, stop=True)
                ob = sm.tile([16, N_FOREST], F32, tag='ob')
                nc.scalar.activation(ob[:, :], o_ps[:, :], AF.Identity,
                                     bias=sb['b2p'], scale=1.0)
                ored = sm.tile([16, 1], F32, tag='ored')
                nc.vector.tensor_reduce(ored[:, :], ob[:, :], mybir.AxisListType.X, OP.add)
                ofin = sm.tile([16, 1], F32, tag='ofin')
                nc.vector.tensor_scalar_mul(ofin[:, :], ored[:, :], 1.0 / N_FOREST)
                nc.sync.dma_start(out=out_d[b, :], in_=ofin[:, 0:1])
    nc.finalize()
    return nc


_NC_CACHE = {}


def kernel(**inputs):
    inp = {k: np.asarray(v) for k, v in inputs.items()}
    H_ = _host_precompute(inp)
    if 'nc' not in _NC_CACHE:
        _NC_CACHE['nc'] = _build_nc()
    nc = _NC_CACHE['nc']
    in_maps = []
    for c in range(NCORES):
        m = {k: np.ascontiguousarray(H_[k]) for k in H_ if k in _H_SHAPES}
        m.update({k: np.ascontiguousarray(v) for k, v in _host_x(inp, (2 * c, 2 * c + 1)).items()})
        in_maps.append(m)
    res = run_bass_kernel_spmd(nc, in_maps, list(range(NCORES)))
    out = np.zeros((B, N_CLASS), np.float32)
    for c in range(NCORES):
        out[2 * c:2 * c + 2] = res.results[c]['out'][:, :N_CLASS]
    return out

